# revision 1
# baseline (speedup 1.0000x reference)
"""Trainium2 Bass kernel for nn_Block_68358699483301 (gnn_message_passing).

Sharding (8 NeuronCores): data-parallel over batch (2) x node-shard (4).
Core c handles batch b=c//4, node rows [384*(c%4), 384*(c%4)+384).

Per-core pipeline (channel-major on chip):
  - neighbor gather via dma_gather from a 256B-padded HBM pos table
  - radial embedding: sin-wrapped periodic edges, d via exp(0.5*ln(r2)),
    exp radial bins on ACT, cosine cutoff as a degree-7 polynomial in
    v=min(r2/9,1) on the vector engine
  - LocalResidual: PE matmuls; groupnorms via block-indicator matmuls
    (partition-group reductions and broadcasts on the PE)
  - probe points; the periodic gaussian proximity kernel is expanded
    trigonometrically into a rank-6-per-probe bilinear form that is folded
    into the attention score matmul (per-probe constant cancels in softmax)
  - AllGather of (z_a, z_v|ones, trig_k) across the 4 cores of a replica
  - dense proximity attention: per-head K=14 score matmuls, 4-way row-tiled
    on the PE; ACT exp (no max subtraction - scores are bounded); attn @
    [va|vv|ones] col-tiled matmuls accumulate over the 12 key chunks in
    PSUM; the ones column yields the softmax denominator for free
  - epilogue matmuls fold head-slot extraction + output projections + the
    final transpose back to node-major
"""

import sys

sys.path.insert(0, "/opt/trn_rl_repo")

import numpy as np
from contextlib import ExitStack

import concourse.bass as bass
import concourse.mybir as mybir
from concourse import bacc, tile
from concourse.bass_utils import run_bass_kernel_spmd

f32 = mybir.dt.float32
bf16 = mybir.dt.bfloat16
i16 = mybir.dt.int16

# ---------------- problem constants (hardcoded) ----------------
B, N, DEG = 2, 1536, 32
ADIM, VDIM, RANK = 128, 32, 32
NH, DH, NPROBE = 16, 8, 2
NB = 8                      # radial bins
R_CUT = 3.0
SIG = R_CUT / NB            # 0.375
R0 = (1.0, 2.0)
EPS = 1e-5
BOX = 12.0
ANG = BOX / (2.0 * np.pi)
R = 384                     # rows per core
NT = 3                      # node tiles per core
NCORES = 8
PI = float(np.pi)
ACOLS = 1161

CKS = np.linspace(0.0, R_CUT, NB).astype(np.float64)  # bin centers

# cut(v) ~= 1 + cos(pi*sqrt(v)) on [0,1], v = min(r2/9, 1); degree-7 poly fit
_vg = np.linspace(0.0, 1.0, 4001)
_tg = 1.0 + np.cos(np.pi * np.sqrt(_vg))
_CH = np.polynomial.chebyshev.Chebyshev.fit(_vg, _tg, 7)
CUTPOLY = _CH.convert(kind=np.polynomial.Polynomial).coef  # ascending, len 8

# gathered layout per rank: cols [0:384] z_a | [384:768] rows0:100 zv1 | [768:1152] rows0:12 trig_k
RBLK = 1152


# ---------------- host-side weight packing ----------------
class _Pack:
    def __init__(self):
        self.cols = []
        self.pos = 0
        self.sl = {}

    def add(self, name, arr):
        arr = np.asarray(arr, np.float32)
        r, c = arr.shape
        assert r <= 128
        a = np.zeros((128, c), np.float32)
        a[:r] = arr
        self.sl[name] = (self.pos, c, r)
        self.cols.append(a)
        self.pos += c

    def build(self):
        return np.ascontiguousarray(np.concatenate(self.cols, axis=1))


def _blockdiag3(m):
    r, c = m.shape
    out = np.zeros((3 * r, 3 * c), np.float32)
    for e in range(3):
        out[e * r:(e + 1) * r, e * c:(e + 1) * c] = m
    return out


def make_wpack(inp):
    g = {k: np.asarray(v, np.float32) for k, v in inp.items() if k != "neighbors"}
    P = _Pack()
    P.add("ident", np.eye(128, dtype=np.float32))
    P.add("emb_a", g["emb_a_w"] * 0.5)                         # (8,128)
    P.add("emb_a_b", g["emb_a_b"][:, None])                    # (128,1)
    P.add("emb_v", _blockdiag3(g["emb_v_w"].T) * (ANG * 0.5))  # (24,96)
    P.add("la1", g["la1_w"])
    P.add("la1_b", g["la1_b"][:, None])
    P.add("la2", g["la2_w"])
    P.add("la2_b", g["la2_b"][:, None])
    P.add("la3", g["la3_w"])
    P.add("zb3", (g["la3_b"] + g["avp_ao_b"])[:, None])
    P.add("avp_a", g["avp_a_w"])                               # (128,32)
    P.add("avp_a_b", g["avp_a_b"][:, None])                    # (32,1)
    P.add("avp_ao_w_", g["avp_ao_w"])                          # (32,128)
    P.add("Wavp_v", _blockdiag3(g["avp_v_w"].T))               # (96,96)
    e3s = np.zeros((96, 32), np.float32)
    for e in range(3):
        e3s[e * 32 + np.arange(32), np.arange(32)] = 1.0
    P.add("E3sum", e3s)
    P.add("E3rep", e3s.T.copy())
    P.add("Wavp_vo", _blockdiag3(g["avp_vo_w"].T))
    P.add("Wlin_v", _blockdiag3(g["lin_v_w"].T))
    g16 = np.zeros((128, 8), np.float32)
    g16[np.arange(128), np.arange(128) // 16] = 1.0 / 16.0
    P.add("G16", g16)
    P.add("H16", (g16 > 0).astype(np.float32).T.copy())
    P.add("gna_g", g["gna_g"][:, None])
    P.add("gna_b", g["gna_b"][:, None])
    g12 = np.zeros((96, 8), np.float32)
    for e in range(3):
        for c in range(32):
            g12[e * 32 + c, c // 4] = 0.25   # mean over 4 ch of sum over 3 coords
    P.add("G12v", g12)
    P.add("H12v", (g12 > 0).astype(np.float32).T.copy())
    gnv = np.zeros((96, 1), np.float32)
    for e in range(3):
        gnv[e * 32:(e + 1) * 32, 0] = g["gnv_g"]
    P.add("gnv", gnv)
    # probe matmuls (100,12): rows 0:96 blockdiag pq/pk, row 96 zero (ones row
    # of zv1), rows 97:100 pos selector; cols [0:6]==[6:12] (cos/sin args)
    for nm, w_ in (("Wp12q", g["pq_w"]), ("Wp12k", g["pk_w"])):
        m6 = np.zeros((100, 6), np.float32)
        for e in range(3):
            for c in range(32):
                for p in range(2):
                    m6[e * 32 + c, e * 2 + p] = w_[p, c]
        for e in range(3):
            for p in range(2):
                m6[97 + e, e * 2 + p] = 1.0
        m12 = np.concatenate([m6, m6], axis=1)                 # (100,12)
        # cos args (cols 0:6): +pi/2 folded in via the ones row (scaled so the
        # later *(2/ANG) yields exactly +pi/2)
        m12[96, 0:6] = (np.pi / 2.0) * (ANG / 2.0)
        P.add(nm, m12)
    # attention slot weights: head h=4g+j lives in cols g*128 + j*32 + [..]
    wq_s = np.zeros((128, 512), np.float32)
    wk_s = np.zeros((128, 512), np.float32)
    wva_s = np.zeros((128, 512), np.float32)
    for gg in range(4):
        for j in range(4):
            h = 4 * gg + j
            for d in range(DH):
                col = gg * 128 + j * 32 + d
                wq_s[:, col] = g["wq"][:, h * DH + d] / np.sqrt(DH)
                wk_s[:, col] = g["wk"][:, h * DH + d]
                # V slot layout: [ones(0), va(1:9), vv(9:15)]
                wva_s[:, gg * 128 + j * 32 + 1 + d] = g["wva"][:, h * DH + d]
    P.add("wq_s", wq_s)
    P.add("wk_s", wk_s)
    P.add("wva_s", wva_s)
    # trig selectors; trig rows: m<6 cos (e*2+p), m>=6 sin (6+e*2+p)
    selq = np.zeros((12, 512), np.float32)
    selk = np.zeros((12, 512), np.float32)
    for gg in range(4):
        p = gg // 2
        gam = ANG * ANG / (4.0 * R0[p] * R0[p])
        for j in range(4):
            for mm in range(6):
                c = mm % 3
                fn = mm // 3
                trow = fn * 6 + c * 2 + p
                col = gg * 128 + j * 32 + 8 + mm
                selq[trow, col] = 1.0
                selk[trow, col] = gam
    P.add("Selq", selq)
    P.add("Selk", selk)
    # vv slots + ones column, contracted against [z_v; ones] (97 rows)
    wvv_s = np.zeros((97, 512), np.float32)
    for gg in range(4):
        for j in range(4):
            h = 4 * gg + j
            for c2 in range(2):
                for e in range(3):
                    col = gg * 128 + j * 32 + 9 + c2 * 3 + e
                    wvv_s[e * 32:(e + 1) * 32, col] = g["wvv"][h * 2 + c2, :]
            wvv_s[96, gg * 128 + j * 32 + 0] = 1.0
    P.add("wvv_s", wvv_s)
    # denom broadcast selector: recip row h -> slots j*32+[0:15] of its group
    bsel = np.zeros((128, 512), np.float32)
    for gg in range(4):
        for j in range(4):
            bsel[32 * gg + j, gg * 128 + j * 32: gg * 128 + j * 32 + 15] = 1.0
    P.add("Bsel", bsel)
    # epilogue
    woa_s = np.zeros((128, 512), np.float32)
    for gg in range(4):
        for j in range(4):
            h = 4 * gg + j
            for d in range(DH):
                woa_s[j * 32 + 1 + d, gg * 128:(gg + 1) * 128] = g["wo_a"][h * DH + d, :]
    P.add("woa_s", woa_s)
    P.add("woa_b", g["wo_a_b"][:, None])
    wov_s = np.zeros((128, 384), np.float32)
    for gg in range(4):
        for j in range(4):
            h = 4 * gg + j
            for c2 in range(2):
                for e in range(3):
                    row = j * 32 + 9 + c2 * 3 + e
                    wov_s[row, gg * 96 + e * 32: gg * 96 + e * 32 + 32] = g["wo_v"][:, h * 2 + c2]
    P.add("wov_s", wov_s)
    ckb = np.zeros((128, NB), np.float32)
    for k in range(NB):
        ckb[:, k] = -CKS[k] / SIG
    P.add("ckb", ckb)
    P.add("c1em6", np.full((128, 1), 1e-6, np.float32))
    P.add("ceps", np.full((128, 1), EPS, np.float32))
    return P


# ---------------- device graph ----------------
def build_nc(wsl, wtot, reps=1):
    nc = bacc.Bacc("TRN2", target_bir_lowering=False, debug=False, num_devices=NCORES)
    W_d = nc.dram_tensor("wpack", [128, wtot], f32, kind="ExternalInput")
    A_d = nc.dram_tensor("apack", [128, ACOLS], f32, kind="ExternalInput")
    IDX_d = nc.dram_tensor("idxpack", [128, 768], i16, kind="ExternalInput")
    TBL_d = nc.dram_tensor("ptable", [N, 64], f32, kind="ExternalInput")
    OUT_d = nc.dram_tensor("outp", [R, 224], f32, kind="ExternalOutput")

    with tile.TileContext(nc) as tc:
        with ExitStack() as ctx:
            main = ctx.enter_context(tc.tile_pool(name="main", bufs=1))
            dram = ctx.enter_context(tc.tile_pool(name="dramp", bufs=1, space="DRAM"))

            W = main.tile([128, wtot], f32, tag="W")
            A = main.tile([128, ACOLS], f32, tag="A")
            IDX = main.tile([128, 768], i16, tag="IDX")
            nc.gpsimd.dma_start(out=W[:, :], in_=W_d[:, :])
            nc.gpsimd.dma_start(out=A[:, :], in_=A_d[:, :])
            nc.gpsimd.dma_start(out=IDX[:, :], in_=IDX_d[:, :])

            def wap(name, rows=None, cols=None):
                off, c, r = wsl[name]
                rr = r if rows is None else rows
                if cols is None:
                    return W[0:rr, off:off + c]
                a, b2 = cols
                return W[0:rr, off + a:off + b2]

            IDENT = wap("ident")

            zeros_sb = main.tile([128, 384], f32, tag="zeros_sb")
            nc.vector.memset(zeros_sb[:, :], 0.0)

            embaT = main.tile([8, 384], f32, tag="embaT")
            embvT = main.tile([24, 384], f32, tag="embvT")

            for _rep in range(reps):
                # ---------------- neighbor gather + embedding ----------------
                with tc.tile_pool(name="embp", bufs=1) as embp, \
                     tc.tile_pool(name="embps", bufs=2, space="PSUM") as embps:
                    G = embp.tile([128, 96, 64], f32, tag="G")
                    import os as _os
                    if _os.environ.get("KDBG_NOGATHER", "0") == "1":
                        for _t in range(8):
                            nc.gpsimd.dma_start(out=G[:, _t * 12:(_t + 1) * 12, :],
                                                in_=TBL_d[:, :].rearrange(
                                                    "(w p) e -> p w e", p=128))
                    else:
                        # single_packet only supports <=64 descs/engine (1024 idxs).
                        # One gather per node-tile so edge processing of tile t can
                        # start while tile t+1 is still gathering.
                        for gt in range(NT):
                            nc.gpsimd.dma_gather(
                                out_ap=G[:, gt * 32:(gt + 1) * 32, :],
                                in_ap=TBL_d[:, :],
                                idxs_ap=IDX[:, gt * 256:(gt + 1) * 256],
                                num_idxs=4096,
                                num_idxs_reg=4096,
                                elem_size=64,
                                single_packet=False,
                            )
                    SARG = embp.tile([128, 96, 3], f32, tag="SARG")
                    for t in range(NT):
                        for c in range(3):
                            nc.vector.tensor_scalar(
                                out=SARG[:, t * 32:(t + 1) * 32, c],
                                in0=G[:, t * 32:(t + 1) * 32, c],
                                scalar1=A[:, 768 + t * 3 + c: 769 + t * 3 + c],
                                scalar2=1.0 / ANG,
                                op0=mybir.AluOpType.subtract,
                                op1=mybir.AluOpType.mult,
                            )
                    WARG = embp.tile([128, 96, 3], f32, tag="WARG")
                    nc.vector.add_range_wrap(out=WARG[:, :, :], in_=SARG[:, :, :],
                                             shift=0.0, bound=PI, period=2 * PI)
                    S = embp.tile([128, 96, 3], f32, tag="S")
                    nc.scalar.activation(S[:, :, :], WARG[:, :, :],
                                         mybir.ActivationFunctionType.Sin)
                    SQ = embp.tile([128, 96, 3], f32, tag="SQ")
                    nc.vector.tensor_tensor(out=SQ[:, :, :], in0=S[:, :, :], in1=S[:, :, :],
                                            op=mybir.AluOpType.mult)
                    R2S = embp.tile([128, 96], f32, tag="R2S")
                    nc.vector.reduce_sum(R2S[:, :], SQ[:, :, :], axis=mybir.AxisListType.X)
                    LNT = embp.tile([128, 96], f32, tag="LNT")
                    nc.scalar.activation(LNT[:, :], R2S[:, :], mybir.ActivationFunctionType.Ln,
                                         bias=wap("c1em6"), scale=float(ANG * ANG))
                    D = embp.tile([128, 96], f32, tag="D")
                    nc.scalar.activation(D[:, :], LNT[:, :], mybir.ActivationFunctionType.Exp,
                                         scale=0.5)
                    V1 = embp.tile([128, 96], f32, tag="V1")
                    nc.vector.tensor_scalar(
                        out=V1[:, :], in0=R2S[:, :],
                        scalar1=float(ANG * ANG / 9.0), scalar2=1.0,
                        op0=mybir.AluOpType.mult, op1=mybir.AluOpType.min)
                    CUT = embp.tile([128, 96], f32, tag="CUT")
                    TMPP = embp.tile([128, 96], f32, tag="TMPP")
                    cc = CUTPOLY
                    nc.vector.tensor_scalar(
                        out=CUT[:, :], in0=V1[:, :], scalar1=float(cc[7]), scalar2=float(cc[6]),
                        op0=mybir.AluOpType.mult, op1=mybir.AluOpType.add)
                    for kk in range(5, -1, -1):
                        nc.vector.tensor_tensor(out=TMPP[:, :], in0=CUT[:, :], in1=V1[:, :],
                                                op=mybir.AluOpType.mult)
                        nc.vector.tensor_scalar(
                            out=CUT[:, :], in0=TMPP[:, :], scalar1=float(cc[kk]), scalar2=0.0,
                            op0=mybir.AluOpType.add, op1=mybir.AluOpType.add)
                    BQ = embp.tile([128, NB, 96], f32, tag="BQ")
                    for k in range(NB):
                        nc.scalar.activation(BQ[:, k, :], D[:, :],
                                             mybir.ActivationFunctionType.Square,
                                             bias=wap("ckb", rows=128, cols=(k, k + 1)),
                                             scale=float(1.0 / SIG))
                    E8 = embp.tile([128, NB, 96], f32, tag="E8")
                    nc.scalar.activation(E8[:, :, :], BQ[:, :, :],
                                         mybir.ActivationFunctionType.Exp, scale=-0.5)
                    XT = embp.tile([128, NB, 96], f32, tag="XT")
                    for k in range(NB):
                        nc.vector.tensor_tensor(out=XT[:, k, :], in0=E8[:, k, :], in1=CUT[:, :],
                                                op=mybir.AluOpType.mult)
                    EA = embp.tile([128, 3, 8], f32, tag="EA")
                    nc.vector.reduce_sum(
                        EA[:, :, :].rearrange("p t k -> p k t"),
                        XT[:, :, :].rearrange("p k (t j) -> p k t j", j=32),
                        axis=mybir.AxisListType.X)
                    EV = embp.tile([128, 3, 24], f32, tag="EV")
                    MV = embp.tile([128, NB, 96], f32, tag="MV")
                    for c in range(3):
                        for k in range(NB):
                            nc.vector.tensor_tensor(out=MV[:, k, :], in0=XT[:, k, :],
                                                    in1=S[:, :, c], op=mybir.AluOpType.mult)
                        nc.vector.reduce_sum(
                            EV[:, :, c * 8:(c + 1) * 8].rearrange("p t k -> p k t"),
                            MV[:, :, :].rearrange("p k (t j) -> p k t j", j=32),
                            axis=mybir.AxisListType.X)
                    for t in range(NT):
                        pstr = embps.tile([8, 128], f32, tag="pstr")
                        nc.tensor.matmul(pstr[:, :], EA[:, t, :], IDENT[:, 0:128],
                                         is_transpose=True)
                        nc.vector.tensor_copy(embaT[:, t * 128:(t + 1) * 128], pstr[:, :])
                        pstr2 = embps.tile([24, 128], f32, tag="pstr2")
                        nc.tensor.matmul(pstr2[:, :], EV[:, t, :], IDENT[:, 0:128],
                                         is_transpose=True)
                        nc.vector.tensor_copy(embvT[:, t * 128:(t + 1) * 128], pstr2[:, :])

                # ---------------- pre-attention local residual ----------------
                zv1 = main.tile([100, 384], f32, tag="zv1")
                z_aT = main.tile([128, 384], f32, tag="z_aT")
                TRIG = main.tile([12, 768], f32, tag="TRIG")
                with tc.tile_pool(name="prep", bufs=1) as prep, \
                     tc.tile_pool(name="preps", bufs=1, space="PSUM") as pp:
                    ps_ya = pp.tile([128, 384], f32, tag="pbig", bufs=1, name="ps_ya")
                    nc.tensor.matmul(ps_ya[:, :], wap("emb_a"), embaT[:, :])
                    y_aT = prep.tile([128, 384], f32, tag="y_aT")
                    nc.vector.scalar_tensor_tensor(
                        out=y_aT[:, :], in0=ps_ya[:, :], scalar=wap("emb_a_b"),
                        in1=A[:, 0:384], op0=mybir.AluOpType.add, op1=mybir.AluOpType.add)
                    ps_yv = pp.tile([96, 384], f32, tag="pmed", bufs=2, name="ps_yv")
                    nc.tensor.matmul(ps_yv[:, :], wap("emb_v"), embvT[:, :])
                    y_vT = prep.tile([96, 384], f32, tag="y_vT")
                    nc.vector.tensor_tensor(out=y_vT[:, :], in0=ps_yv[:, :],
                                            in1=A[0:96, 384:768], op=mybir.AluOpType.add)
                    h1 = prep.tile([128, 384], f32, tag="h1")
                    ps_h = pp.tile([128, 384], f32, tag="pbig", bufs=1, name="ps_h")
                    nc.tensor.matmul(ps_h[:, :], wap("la1"), y_aT[:, :])
                    t1 = prep.tile([128, 384], f32, tag="t1")
                    nc.vector.tensor_scalar(out=t1[:, :], in0=ps_h[:, :],
                        scalar1=wap("la1_b"), scalar2=0.0,
                        op0=mybir.AluOpType.add, op1=mybir.AluOpType.add)
                    nc.vector.scalar_tensor_tensor(
                        out=h1[:, :], in0=t1[:, :], scalar=0.2, in1=t1[:, :],
                        op0=mybir.AluOpType.mult, op1=mybir.AluOpType.max)
                    h2 = prep.tile([128, 384], f32, tag="h2")
                    ps_h2 = pp.tile([128, 384], f32, tag="pbig", bufs=1, name="ps_h2")
                    nc.tensor.matmul(ps_h2[:, :], wap("la2"), h1[:, :])
                    t2 = prep.tile([128, 384], f32, tag="t2")
                    nc.vector.tensor_scalar(out=t2[:, :], in0=ps_h2[:, :],
                        scalar1=wap("la2_b"), scalar2=0.0,
                        op0=mybir.AluOpType.add, op1=mybir.AluOpType.add)
                    nc.vector.scalar_tensor_tensor(
                        out=h2[:, :], in0=t2[:, :], scalar=0.2, in1=t2[:, :],
                        op0=mybir.AluOpType.mult, op1=mybir.AluOpType.max)
                    ps_za = pp.tile([128, 384], f32, tag="pza", name="ps_za")
                    nc.tensor.matmul(ps_za[:, :], wap("la3"), h2[:, :], start=True, stop=False)
                    ps_ar = pp.tile([32, 384], f32, tag="psmall", bufs=2, name="ps_ar")
                    nc.tensor.matmul(ps_ar[:, :], wap("avp_a"), y_aT[:, :])
                    a_r = prep.tile([32, 384], f32, tag="a_r")
                    nc.vector.tensor_scalar(out=a_r[:, :], in0=ps_ar[:, :],
                        scalar1=wap("avp_a_b"), scalar2=0.0,
                        op0=mybir.AluOpType.add, op1=mybir.AluOpType.add)
                    ps_vr = pp.tile([96, 384], f32, tag="pmed", bufs=2, name="ps_vr")
                    nc.tensor.matmul(ps_vr[:, :], wap("Wavp_v"), y_vT[:, :])
                    vr = prep.tile([96, 384], f32, tag="vr")
                    nc.vector.tensor_copy(vr[:, :], ps_vr[:, :])
                    vr2 = prep.tile([96, 384], f32, tag="vr2")
                    nc.vector.tensor_tensor(out=vr2[:, :], in0=vr[:, :], in1=vr[:, :],
                                            op=mybir.AluOpType.mult)
                    ps_sq = pp.tile([32, 384], f32, tag="psmall", bufs=2, name="ps_sq")
                    nc.tensor.matmul(ps_sq[:, :], wap("E3sum"), vr2[:, :])
                    t_ar = prep.tile([32, 384], f32, tag="t_ar")
                    nc.vector.tensor_tensor(out=t_ar[:, :], in0=a_r[:, :], in1=ps_sq[:, :],
                                            op=mybir.AluOpType.mult)
                    nc.tensor.matmul(ps_za[:, :], wap("avp_ao_w_"), t_ar[:, :],
                                     start=False, stop=True)
                    ga = prep.tile([128, 384], f32, tag="ga")
                    nc.vector.tensor_scalar(out=ga[:, :], in0=ps_za[:, :],
                        scalar1=wap("zb3"), scalar2=0.0,
                        op0=mybir.AluOpType.add, op1=mybir.AluOpType.add)
                    ps_arep = pp.tile([96, 384], f32, tag="pmed", bufs=2, name="ps_arep")
                    nc.tensor.matmul(ps_arep[:, :], wap("E3rep"), a_r[:, :])
                    w_in = prep.tile([96, 384], f32, tag="w_in")
                    nc.vector.tensor_tensor(out=w_in[:, :], in0=ps_arep[:, :], in1=vr[:, :],
                                            op=mybir.AluOpType.mult)
                    ps_zv = pp.tile([96, 384], f32, tag="pza", name="ps_zv")
                    nc.tensor.matmul(ps_zv[:, :], wap("Wlin_v"), y_vT[:, :], start=True, stop=False)
                    nc.tensor.matmul(ps_zv[:, :], wap("Wavp_vo"), w_in[:, :], start=False, stop=True)
                    gv = prep.tile([96, 384], f32, tag="gv")
                    nc.vector.tensor_copy(gv[:, :], ps_zv[:, :])

                    # scalar groupnorm
                    ps8 = pp.tile([8, 384], f32, tag="psmall", bufs=2, name="ps8")
                    nc.tensor.matmul(ps8[:, :], wap("G16"), ga[:, :])
                    mu8 = prep.tile([8, 384], f32, tag="mu8")
                    nc.vector.tensor_copy(mu8[:, :], ps8[:, :])
                    ps_mub = pp.tile([128, 384], f32, tag="pbig", bufs=1, name="ps_mub")
                    nc.tensor.matmul(ps_mub[:, :], wap("H16"), mu8[:, :])
                    xc = prep.tile([128, 384], f32, tag="xc")
                    nc.vector.tensor_tensor(out=xc[:, :], in0=ga[:, :], in1=ps_mub[:, :],
                                            op=mybir.AluOpType.subtract)
                    sqx = prep.tile([128, 384], f32, tag="sqx")
                    nc.vector.tensor_tensor(out=sqx[:, :], in0=xc[:, :], in1=xc[:, :],
                                            op=mybir.AluOpType.mult)
                    ps8b = pp.tile([8, 384], f32, tag="psmall", bufs=2, name="ps8b")
                    nc.tensor.matmul(ps8b[:, :], wap("G16"), sqx[:, :])
                    lnv = prep.tile([8, 384], f32, tag="lnv")
                    nc.scalar.activation(lnv[:, :], ps8b[:, :], mybir.ActivationFunctionType.Ln,
                                         bias=wap("ceps", rows=8))
                    rstd8 = prep.tile([8, 384], f32, tag="rstd8")
                    nc.scalar.activation(rstd8[:, :], lnv[:, :], mybir.ActivationFunctionType.Exp,
                                         scale=-0.5)
                    ps_rb = pp.tile([128, 384], f32, tag="pbig", bufs=1, name="ps_rb")
                    nc.tensor.matmul(ps_rb[:, :], wap("H16"), rstd8[:, :])
                    xn = prep.tile([128, 384], f32, tag="xn")
                    nc.vector.tensor_tensor(out=xn[:, :], in0=xc[:, :], in1=ps_rb[:, :],
                                            op=mybir.AluOpType.mult)
                    nc.vector.affine_then_add(out=z_aT[:, :], in0=xn[:, :], in1=y_aT[:, :],
                                              scale=wap("gna_g"), bias=wap("gna_b"))
                    # vector groupnorm
                    sqv = prep.tile([96, 384], f32, tag="sqv")
                    nc.vector.tensor_tensor(out=sqv[:, :], in0=gv[:, :], in1=gv[:, :],
                                            op=mybir.AluOpType.mult)
                    ps8c = pp.tile([8, 384], f32, tag="psmall", bufs=2, name="ps8c")
                    nc.tensor.matmul(ps8c[:, :], wap("G12v"), sqv[:, :])
                    lnv2 = prep.tile([8, 384], f32, tag="lnv2")
                    nc.scalar.activation(lnv2[:, :], ps8c[:, :], mybir.ActivationFunctionType.Ln,
                                         bias=wap("ceps", rows=8))
                    rstd8v = prep.tile([8, 384], f32, tag="rstd8v")
                    nc.scalar.activation(rstd8v[:, :], lnv2[:, :],
                                         mybir.ActivationFunctionType.Exp, scale=-0.5)
                    ps_rbv = pp.tile([96, 384], f32, tag="pmed", bufs=2, name="ps_rbv")
                    nc.tensor.matmul(ps_rbv[:, :], wap("H12v"), rstd8v[:, :])
                    xnv = prep.tile([96, 384], f32, tag="xnv")
                    nc.vector.tensor_tensor(out=xnv[:, :], in0=gv[:, :], in1=ps_rbv[:, :],
                                            op=mybir.AluOpType.mult)
                    nc.vector.affine_then_add(out=zv1[0:96, :], in0=xnv[:, :], in1=y_vT[:, :],
                                              scale=wap("gnv"), bias=0.0)
                    nc.gpsimd.dma_start(out=zv1[96:100, :], in_=A_d[0:4, 777:1161])

                    # probes + trig features
                    for half, wnm in ((0, "Wp12q"), (1, "Wp12k")):
                        ps_p = pp.tile([12, 384], f32, tag="psmall", bufs=2, name=f"ps_p{half}")
                        nc.tensor.matmul(ps_p[:, :], wap(wnm), zv1[:, :])
                        ta = pp.tile([12, 384], f32, tag="ptrigA", name=f"ta{half}")
                        tb = pp.tile([12, 384], f32, tag="ptrigB", name=f"tb{half}")
                        nc.vector.tensor_scalar(out=ta[:, :], in0=ps_p[:, :],
                                                scalar1=float(2.0 / ANG), scalar2=0.0,
                                                op0=mybir.AluOpType.mult, op1=mybir.AluOpType.add)
                        for _ in range(4):
                            nc.vector.add_range_wrap(out=tb[:, :], in_=ta[:, :],
                                                     shift=0.0, bound=PI, period=2 * PI)
                            ta, tb = tb, ta
                        nc.scalar.activation(TRIG[:, half * 384:(half + 1) * 384], ta[:, :],
                                             mybir.ActivationFunctionType.Sin)

                # ---------------- collective: allgather z/trig ----------------
                bounce_in = dram.tile([128, RBLK], f32, tag="bounce_in")
                bounce_out = dram.tile([4, 128, RBLK], f32, tag="bounce_out")
                nc.gpsimd.dma_start(out=bounce_in[:, 0:384], in_=z_aT[:, :])
                nc.gpsimd.dma_start(out=bounce_in[0:100, 384:768], in_=zv1[:, :])
                nc.gpsimd.dma_start(out=bounce_in[100:128, 384:768], in_=zeros_sb[0:28, :])
                nc.gpsimd.dma_start(out=bounce_in[0:12, 768:1152], in_=TRIG[:, 384:768])
                nc.gpsimd.dma_start(out=bounce_in[12:128, 768:1152], in_=zeros_sb[0:116, :])
                import os as _os
                if _os.environ.get("KDBG_NOCC", "0") == "1":
                    for r in range(4):
                        nc.gpsimd.dma_start(out=bounce_out[r, :, :], in_=bounce_in[:, :])
                else:
                    nc.gpsimd.collective_compute(
                        "AllGather",
                        mybir.AluOpType.bypass,
                        ins=[bounce_in[:, :].opt()],
                        outs=[bounce_out[:, :, :].opt()],
                        replica_groups=[[0, 1, 2, 3], [4, 5, 6, 7]],
                    )
                GSB = main.tile([128, 4 * RBLK], f32, tag="GSB")
                for r in range(4):
                    nc.gpsimd.dma_start(out=GSB[:, r * RBLK:(r + 1) * RBLK],
                                      in_=bounce_out[r, :, :])

                # ---------------- build V / augK / augQ ----------------
                V_sb = main.tile([128, 12 * 512], bf16, tag="V_sb")
                augK = [main.tile([128, 1536], f32, tag=f"augK{g}", name=f"augK{g}")
                        for g in range(4)]
                augQ = [main.tile([128, 384], f32, tag=f"augQ{g}", name=f"augQ{g}")
                        for g in range(4)]
                with tc.tile_pool(name="bldps", bufs=2, space="PSUM") as bps:
                    for g in range(4):
                        ps_q = bps.tile([128, 384], f32, tag="ps_q")
                        nc.tensor.matmul(ps_q[:, :], wap("wq_s", cols=(g * 128, (g + 1) * 128)),
                                         z_aT[:, :], start=True, stop=False)
                        nc.tensor.matmul(ps_q[:, :], wap("Selq", cols=(g * 128, (g + 1) * 128)),
                                         TRIG[:, 0:384], start=False, stop=True)
                        nc.vector.tensor_copy(augQ[g][:, :], ps_q[:, :])
                    for kc in range(12):
                        r, lc = kc // 3, kc % 3
                        za_ch = GSB[:, r * RBLK + lc * 128: r * RBLK + lc * 128 + 128]
                        zv_ch = GSB[0:97, r * RBLK + 384 + lc * 128: r * RBLK + 384 + lc * 128 + 128]
                        ps_v = bps.tile([128, 512], f32, tag="ps_v")
                        nc.tensor.matmul(ps_v[:, :], za_ch, wap("wva_s"), start=True, stop=False)
                        nc.tensor.matmul(ps_v[:, :], zv_ch, wap("wvv_s"), start=False, stop=True)
                        nc.vector.tensor_copy(V_sb[:, kc * 512:(kc + 1) * 512], ps_v[:, :])
                    for g in range(4):
                        for r in range(4):
                            ps_k = bps.tile([128, 384], f32, tag="ps_k")
                            nc.tensor.matmul(ps_k[:, :],
                                             wap("wk_s", cols=(g * 128, (g + 1) * 128)),
                                             GSB[:, r * RBLK: r * RBLK + 384],
                                             start=True, stop=False)
                            nc.tensor.matmul(ps_k[:, :],
                                             wap("Selk", cols=(g * 128, (g + 1) * 128)),
                                             GSB[0:12, r * RBLK + 768: r * RBLK + 1152],
                                             start=False, stop=True)
                            nc.vector.tensor_copy(augK[g][:, r * 384:(r + 1) * 384], ps_k[:, :])

                # ---------------- attention main loop ----------------
                AVsb = main.tile([128, 1536], f32, tag="AVsb")
                with tc.tile_pool(name="avpool", bufs=1, space="PSUM") as avpool:
                    avps = [avpool.tile([128, 384], f32, tag=f"avps{g}", name=f"avps{g}")
                            for g in range(4)]
                    # one full-bank zero matmul opens each accumulation group (and
                    # zeroes the slot-gap partitions the per-head matmuls never touch)
                    for g in range(4):
                        nc.tensor.matmul(avps[g][:, :], zeros_sb[0:1, 0:128],
                                         zeros_sb[0:1, 0:384], start=True, stop=False)
                    with tc.tile_pool(name="scpool", bufs=1, space="PSUM") as scpool, \
                         tc.tile_pool(name="expp", bufs=5) as expp:
                        for kc in range(12):
                            st_ps = scpool.tile([128, 2048], f32, tag="st_ps")
                            exps = []
                            for g in range(4):
                                for j in range(4):
                                    nc.tensor.matmul(
                                        st_ps[:, j * 512: j * 512 + 384],
                                        augK[g][j * 32: j * 32 + 14, kc * 128:(kc + 1) * 128],
                                        augQ[g][j * 32: j * 32 + 14, :],
                                        start=True, stop=True,
                                        tile_position=(32 * j, 0),
                                    )
                                exp_t = expp.tile([128, 4, 384], bf16, tag="exp_t")
                                nc.scalar.activation(
                                    exp_t[:, :, :],
                                    st_ps[:, :].rearrange("p (h q) -> p h q", q=512)[:, :, 0:384],
                                    mybir.ActivationFunctionType.Exp)
                                exps.append(exp_t)
                            for g in range(4):
                                for j in range(4):
                                    base = kc * 512 + g * 128 + j * 32
                                    nc.tensor.matmul(
                                        avps[g][32 * j: 32 * j + 15, :],
                                        V_sb[:, base: base + 15],
                                        exps[g][:, j, :],
                                        start=False, stop=False,
                                        tile_position=(0, 32 * j),
                                    )
                    for g in range(4):
                        # group-close only: N=1 keeps the tail short (col 0 was
                        # written by the kc=0 matmuls, so no pending-zero is touched)
                        nc.tensor.matmul(avps[g][:, 0:1], zeros_sb[0:1, 0:128],
                                         zeros_sb[0:1, 0:1], start=False, stop=True)
                    for g in range(4):
                        nc.vector.tensor_copy(AVsb[:, g * 384:(g + 1) * 384], avps[g][:, :])

                # ---------------- normalize + epilogue + output ----------------
                den128 = main.tile([128, 384], f32, tag="den128")
                rec128 = main.tile([128, 384], f32, tag="rec128")
                nc.vector.memset(den128[:, :], 1.0)
                for g in range(4):
                    nc.gpsimd.dma_start(out=den128[32 * g:32 * g + 4, :],
                                      in_=AVsb[0:128:32, g * 384:(g + 1) * 384])
                nc.vector.reciprocal(rec128[:, :], den128[:, :])
                with tc.tile_pool(name="postp", bufs=2) as postp, \
                     tc.tile_pool(name="postps", bufs=1, space="PSUM") as pops:
                    ps_oa = pops.tile([128, 384], f32, tag="ps_oa")
                    ps_ov = pops.tile([96, 384], f32, tag="ps_ov")
                    for g in range(4):
                        ps_bc = pops.tile([128, 384], f32, tag="ps_bc")
                        nc.tensor.matmul(ps_bc[:, :], wap("Bsel", cols=(g * 128, (g + 1) * 128)),
                                         rec128[:, :])
                        avn = postp.tile([128, 384], f32, tag="avn")
                        nc.vector.tensor_tensor(out=avn[:, :], in0=AVsb[:, g * 384:(g + 1) * 384],
                                                in1=ps_bc[:, :], op=mybir.AluOpType.mult)
                        nc.tensor.matmul(ps_oa[:, :], wap("woa_s", cols=(g * 128, (g + 1) * 128)),
                                         avn[:, :], start=(g == 0), stop=(g == 3))
                        nc.tensor.matmul(ps_ov[:, :], wap("wov_s", cols=(g * 96, (g + 1) * 96)),
                                         avn[:, :], start=(g == 0), stop=(g == 3))
                    zaf = postp.tile([128, 384], f32, tag="zaf")
                    nc.vector.scalar_tensor_tensor(
                        out=zaf[:, :], in0=ps_oa[:, :], scalar=wap("woa_b"),
                        in1=z_aT[:, :], op0=mybir.AluOpType.add, op1=mybir.AluOpType.add)
                    zvf = postp.tile([96, 384], f32, tag="zvf")
                    nc.vector.tensor_tensor(out=zvf[:, :], in0=ps_ov[:, :], in1=zv1[0:96, :],
                                            op=mybir.AluOpType.add)
                    for t in range(NT):
                        ps_t1 = pops.tile([128, 128], f32, tag="ps_t1")
                        nc.tensor.matmul(ps_t1[:, :], zaf[:, t * 128:(t + 1) * 128],
                                         IDENT[:, 0:128], is_transpose=True)
                        ps_t2 = pops.tile([128, 96], f32, tag="ps_t2")
                        nc.tensor.matmul(ps_t2[:, :], zvf[:, t * 128:(t + 1) * 128],
                                         IDENT[0:96, 0:96], is_transpose=True)
                        onm = postp.tile([128, 224], f32, tag="onm")
                        nc.vector.tensor_copy(onm[:, 0:128], ps_t1[:, :])
                        nc.vector.tensor_copy(
                            onm[:, 128:224].rearrange("p (c e) -> p e c", e=3),
                            ps_t2[:, :].rearrange("p (e c) -> p e c", e=3))
                        nc.gpsimd.dma_start(out=OUT_d[t * 128:(t + 1) * 128, :], in_=onm[:, :])

    nc.compile()
    return nc


# ---------------- host wrapper ----------------
def prep_in_maps(inputs):
    P = make_wpack(inputs)
    wpack = P.build()
    x_a = np.asarray(inputs["x_a"], np.float32)
    x_v = np.asarray(inputs["x_v"], np.float32)
    pos = np.asarray(inputs["pos_0"], np.float32)
    nbr = np.asarray(inputs["neighbors"], np.int64)
    tables = []
    for b in range(B):
        t = np.zeros((N, 64), np.float32)
        t[:, 0:3] = pos[b]
        tables.append(np.ascontiguousarray(t))
    in_maps = []
    for core in range(NCORES):
        b, s = core // 4, core % 4
        rows = slice(s * R, (s + 1) * R)
        a = np.zeros((128, ACOLS), np.float32)
        a[:, 0:384] = x_a[b, rows].T
        xv = x_v[b, rows]  # (384, 32, 3)
        for e in range(3):
            a[e * 32:(e + 1) * 32, 384:768] = xv[:, :, e].T
        pr = pos[b, rows]  # (384, 3)
        for t in range(NT):
            for c in range(3):
                a[:, 768 + t * 3 + c] = pr[t * 128:(t + 1) * 128, c]
        a[0, 777:1161] = 1.0
        a[1:4, 777:1161] = pr.T
        idx_flat = np.zeros(R * DEG, np.int64)
        for t in range(NT):
            for j in range(DEG):
                T = t * 32 + j
                idx_flat[T * 128:(T + 1) * 128] = nbr[s * R + t * 128: s * R + (t + 1) * 128, j]
        arr16 = np.ascontiguousarray(idx_flat.reshape(768, 16).T).astype(np.int16)
        idxp = np.ascontiguousarray(np.tile(arr16, (8, 1)))
        in_maps.append({
            "wpack": wpack,
            "apack": np.ascontiguousarray(a),
            "idxpack": idxp,
            "ptable": tables[b],
        })
    return in_maps, P.sl, wpack.shape[1]


_NC_CACHE = {}


def get_nc(wsl, wtot, reps=1):
    key = ("nc", wtot, reps)
    if key not in _NC_CACHE:
        _NC_CACHE[key] = build_nc(wsl, wtot, reps=reps)
    return _NC_CACHE[key]


def kernel(**inputs):
    in_maps, wsl, wtot = prep_in_maps(inputs)
    nc = get_nc(wsl, wtot)
    try:
        res = run_bass_kernel_spmd(nc, in_maps, list(range(NCORES)))
    except Exception:
        # one retry: a previously-wedged accelerator can fail the first
        # execution after reset
        res = run_bass_kernel_spmd(nc, in_maps, list(range(NCORES)))
    out = np.zeros((B, N, 224), np.float32)
    for core in range(NCORES):
        b, s = core // 4, core % 4
        out[b, s * R:(s + 1) * R] = res.results[core]["outp"]
    return out

